# revision 1
# baseline (speedup 1.0000x reference)
"""Trainium2 Bass kernel for a hidden-size-1 GRU over M=65536 independent
sequences (T=12 steps, FE=32 features), followed by relu + linear head.

Strategy (data-parallel over 8 NeuronCores, 8192 sequences each):
  - Host pre-transposes each core's x shard to [384, 8192] (feature-major)
    so the gate projection becomes plain W-stationary matmuls with the
    contraction on partitions, and permutes columns so the final output DMA
    is contiguous per partition.
  - Gate projection: 3 accumulating fp32 matmuls per 512-column group with a
    block-diagonal weight [384, 36] producing all 12 timesteps x 3 gates.
  - Gate biases are folded in during the PSUM->SBUF copy (ScalarE activation
    with a per-partition bias column).
  - PE transpose turns gates [36, m] into [m, 36] tiles, packed into a
    [128, 36*64] SBUF layout ((t,gate)-major, block-column minor).
  - The GRU recurrence runs batched over all 8192 sequences per core:
    tiles [128, 64], 6 VectorE + ~5 ScalarE ops per timestep. Recurrent
    weights/biases are baked into instructions as immediates (the kernel is
    JIT-compiled per weight values inside kernel()).
  - relu + linear head: one big relu, then per-output-channel multiply with a
    broadcast lin_w tile and a strided tensor_reduce over T.
"""

import numpy as np

B, N, FE, T, OUT = 32, 2048, 32, 12, 3
M = B * N
NCORES = 8
MC = M // NCORES          # 8192 sequences per core
C = FE * T                # 384 contraction length
NBLK = MC // 128          # 64 column blocks of 128
NGRP = MC // 512          # 16 matmul groups of 512 columns
NQ = 4                    # DMA quarters per chunk
QW = MC // NQ             # 2048 columns per DMA

_COMPILED = None          # (nc, weights_key)


def _build_program(w_hh, b_hh, lin_b):
    """Build the bass program. w_hh/b_hh/lin_b values are baked as immediates."""
    from contextlib import ExitStack

    import concourse.bass as bass
    import concourse.tile as tile
    from concourse import mybir

    f32 = mybir.dt.float32
    AF = mybir.ActivationFunctionType
    wh0, wh1, wh2 = (float(w_hh[i]) for i in range(3))
    bhh2 = float(b_hh[2])

    nc = bass.Bass("TRN2", target_bir_lowering=False, debug=False)

    xt = nc.dram_tensor("xt", [C, MC], f32, kind="ExternalInput").ap()
    w3d = nc.dram_tensor("w3", [C, 12], f32, kind="ExternalInput").ap()
    biasd = nc.dram_tensor("bias36", [12, 3], f32, kind="ExternalInput").ap()
    identd = nc.dram_tensor("ident36", [36, 36], f32, kind="ExternalInput").ap()
    lwd = nc.dram_tensor("lwb", [OUT, 128, T * NBLK], f32, kind="ExternalInput").ap()
    outd = nc.dram_tensor("out", [MC, OUT], f32, kind="ExternalOutput").ap()

    with ExitStack() as ctx:
        tc = ctx.enter_context(tile.TileContext(nc))
        consts = ctx.enter_context(tc.tile_pool(name="consts", bufs=1))
        xpool = ctx.enter_context(tc.tile_pool(name="x", bufs=1))
        gpool = ctx.enter_context(tc.tile_pool(name="g", bufs=1))
        gs_pool = ctx.enter_context(tc.tile_pool(name="gs", bufs=3))
        work = ctx.enter_context(tc.tile_pool(name="work", bufs=2))
        psum_gp = ctx.enter_context(tc.tile_pool(name="pgp", bufs=2, space="PSUM"))
        psum_gt = ctx.enter_context(tc.tile_pool(name="pgt", bufs=2, space="PSUM"))

        # --- constants ---
        w3_sb = consts.tile([128, 3 * 12], f32, tag="w3sb")
        for j in range(3):
            nc.sync.dma_start(
                out=w3_sb[:, j * 12 : (j + 1) * 12],
                in_=w3d[j * 128 : (j + 1) * 128, :],
            )
        bias_sb = consts.tile([12, 3], f32, tag="bias36")
        nc.sync.dma_start(out=bias_sb, in_=biasd)
        ident_sb = consts.tile([36, 36], f32, tag="ident36")
        nc.sync.dma_start(out=ident_sb, in_=identd)
        lw_sb = consts.tile([128, OUT * T * NBLK], f32, tag="lwsb")
        for o in range(OUT):
            nc.sync.dma_start(
                out=lw_sb[:, o * T * NBLK : (o + 1) * T * NBLK], in_=lwd[o]
            )
        bhh2_sb = consts.tile([128, 1], f32, tag="bhh2")
        nc.vector.memset(bhh2_sb, bhh2)

        # --- stream x in: 12 DMAs of [128, 2048] (1 MiB each), chunk-major so
        # chunk j (timesteps 4j..4j+3, t-major row layout) completes early ---
        xs = {}
        for j in range(3):
            for q in range(NQ):
                xs[(j, q)] = xpool.tile(
                    [128, QW], f32, tag=f"x{j}_{q}", name=f"x{j}_{q}"
                )
                nc.sync.dma_start(
                    out=xs[(j, q)],
                    in_=xt[j * 128 : (j + 1) * 128, q * QW : (q + 1) * QW],
                )

        # --- gates: per-chunk matmul + bias + transpose into G [128, 36*64] ---
        # G column layout: (t*3+gate)*NBLK + kblk; chunk j covers tg 12j..12j+12
        G = gpool.tile([128, 36 * NBLK], f32, tag="G")
        G3 = G.rearrange("p (tg k) -> p tg k", tg=36)
        for j in range(3):
            for g in range(NGRP):
                q, off = g // 4, (g % 4) * 512
                gp = psum_gp.tile([12, 512], f32, tag="gp")
                nc.tensor.matmul(
                    gp,
                    lhsT=w3_sb[:, j * 12 : (j + 1) * 12],
                    rhs=xs[(j, q)][:, off : off + 512],
                    start=True,
                    stop=True,
                )
                gs = gs_pool.tile([12, 512], f32, tag="gs")
                nc.scalar.activation(
                    gs,
                    gp,
                    AF.Identity,
                    bias=bias_sb[:, j : j + 1],
                    scale=1.0,
                )
                gt = psum_gt.tile([128, 4 * 12], f32, tag="gt")
                for qq in range(4):
                    nc.tensor.transpose(
                        gt[:, qq * 12 : (qq + 1) * 12],
                        gs[:, qq * 128 : (qq + 1) * 128],
                        ident_sb[:12, :12],
                    )
                nc.scalar.activation(
                    G3[:, j * 12 : (j + 1) * 12, g * 4 : (g + 1) * 4],
                    gt.rearrange("p (q tg) -> p tg q", q=4),
                    AF.Copy,
                    bias=0.0,
                )

        # --- GRU recurrence, batched [128, 64] ---
        # h' = c*n + z*h with c = 1-z = sigmoid(-pre_z). wh*h+G fused via
        # scalar_tensor_tensor so the critical chain is short.
        H = gpool.tile([128, T * NBLK], f32, tag="H")
        K = NBLK
        add, mult = mybir.AluOpType.add, mybir.AluOpType.mult

        def gcol(t, gate):
            return G[:, (t * 3 + gate) * K : (t * 3 + gate + 1) * K]

        for t in range(T):
            hprev = H[:, (t - 1) * K : t * K] if t > 0 else None
            r = work.tile([128, K], f32, tag="r")
            c = work.tile([128, K], f32, tag="c")
            v = work.tile([128, K], f32, tag="v")
            if t == 0:
                nc.scalar.activation(r, gcol(0, 0), AF.Sigmoid)
                nc.scalar.activation(c, gcol(0, 1), AF.Sigmoid, scale=-1.0)
                nc.vector.tensor_scalar_mul(v, r, bhh2)
            else:
                sr = work.tile([128, K], f32, tag="sr")
                sz = work.tile([128, K], f32, tag="sz")
                nc.vector.scalar_tensor_tensor(sr, hprev, wh0, gcol(t, 0), mult, add)
                nc.vector.scalar_tensor_tensor(sz, hprev, wh1, gcol(t, 1), mult, add)
                nc.scalar.activation(r, sr, AF.Sigmoid)
                nc.scalar.activation(c, sz, AF.Sigmoid, scale=-1.0)
                z = work.tile([128, K], f32, tag="z")
                nc.scalar.activation(z, sz, AF.Sigmoid)
                zh = work.tile([128, K], f32, tag="zh")
                nc.vector.tensor_mul(zh, z, hprev)
                u = work.tile([128, K], f32, tag="u")
                nc.scalar.activation(u, hprev, AF.Identity, bias=bhh2_sb, scale=wh2)
                nc.vector.tensor_mul(v, r, u)
            w = work.tile([128, K], f32, tag="w")
            nc.vector.tensor_add(w, v, gcol(t, 2))
            n = work.tile([128, K], f32, tag="n")
            nc.scalar.activation(n, w, AF.Tanh)
            ht = H[:, t * K : (t + 1) * K]
            if t == 0:
                nc.vector.tensor_mul(ht, n, c)
            else:
                cn = work.tile([128, K], f32, tag="cn")
                nc.vector.tensor_mul(cn, n, c)
                nc.vector.tensor_add(ht, cn, zh)

        # --- relu + linear head, split per t-group of 4 so the first two
        # thirds overlap the tail of the recurrence ---
        TG, GW = 3, 4 * K
        R = gpool.tile([128, T * NBLK], f32, tag="R")
        for tg in range(TG):
            nc.scalar.activation(
                R[:, tg * GW : (tg + 1) * GW], H[:, tg * GW : (tg + 1) * GW], AF.Relu
            )
        out_sb = gpool.tile([128, K * OUT], f32, tag="outsb")
        out_v = out_sb.rearrange("p (k o) -> p o k", o=OUT)
        for o in range(OUT):
            accs = []
            for tg in range(TG):
                P = work.tile([128, GW], f32, tag=f"P{tg}", name=f"P{tg}_{o}")
                nc.vector.tensor_mul(
                    P,
                    R[:, tg * GW : (tg + 1) * GW],
                    lw_sb[:, o * T * NBLK + tg * GW : o * T * NBLK + (tg + 1) * GW],
                )
                acc = work.tile([128, K], f32, tag=f"acc{tg}", name=f"acc{tg}_{o}")
                nc.vector.tensor_reduce(
                    acc,
                    P.rearrange("p (t k) -> p k t", t=4),
                    axis=mybir.AxisListType.X,
                    op=mybir.AluOpType.add,
                )
                accs.append(acc)
            s01 = work.tile([128, K], f32, tag="s01", name=f"s01_{o}")
            nc.vector.tensor_add(s01, accs[0], accs[1])
            s012 = work.tile([128, K], f32, tag="s012", name=f"s012_{o}")
            nc.vector.tensor_add(s012, s01, accs[2])
            nc.vector.tensor_scalar_add(out_v[:, o, :], s012, float(lin_b[o]))

        nc.sync.dma_start(
            out=outd.rearrange("(p k) o -> p (k o)", p=128), in_=out_sb
        )

    _split_multi_waits(nc)
    return nc


def _split_multi_waits(nc):
    """Walrus (this build) rejects instructions with more than one sync-wait
    command. Hoist extra waits onto same-engine NoOps placed just before the
    offending instruction — the engine stream blocks on the NoOps first, so
    semantics are identical."""
    from concourse import mybir

    nid = [0]

    def fresh():
        nid[0] += 1
        return f"I-waitsplit-{nid[0]}"

    for bb in nc.main_func.blocks:
        out = []
        for insn in bb.instructions:
            si = insn.sync_info
            if si is not None and si.on_wait and len(si.on_wait) > 1:
                waits = list(si.on_wait)
                for w in waits[:-1]:
                    nop = mybir.InstNoOp(
                        name=fresh(), engine=insn.engine, ins=[], outs=[]
                    )
                    nop.sync_info = mybir.SyncInfo(on_wait=[w], on_update=[])
                    out.append(nop)
                insn.sync_info = mybir.SyncInfo(
                    on_wait=[waits[-1]], on_update=list(si.on_update or [])
                )
            out.append(insn)
        bb.instructions = out


def _host_prep(x, w_ih, w_hh, b_ih, b_hh, lin_w):
    """Build per-core xt shards and the shared constant arrays."""
    # t-major feature rows: row c' = t*FE + f, so DMA chunk j = timesteps 4j..4j+3
    xflat = np.ascontiguousarray(
        x.reshape(M, FE, T).transpose(0, 2, 1).reshape(M, C), dtype=np.float32
    )
    jj = np.arange(MC)
    rows = (jj % 128) * (MC // 128) + (jj // 128)

    # Per-chunk block-diagonal gate weights:
    # W3[t*FE+f, (t%4)*3+g] = w_ih[g, f]
    w3 = np.zeros((T, FE, 12), dtype=np.float32)
    for t in range(T):
        w3[t, :, (t % 4) * 3 : (t % 4) * 3 + 3] = w_ih.T
    w3 = np.ascontiguousarray(w3.reshape(C, 12))

    # bias36[tg_local, chunk]: bias for global tg = 12*chunk + tg_local
    bias36 = np.zeros((12, 3), dtype=np.float32)
    for t in range(T):
        for g in range(3):
            bias36[(t % 4) * 3 + g, t // 4] = b_ih[g] + (b_hh[g] if g < 2 else 0.0)

    ident36 = np.eye(36, dtype=np.float32)

    # lwb[o] broadcast tile: col t*NBLK + k = lin_w[o, t]
    lwb = np.empty((OUT, 128, T * NBLK), dtype=np.float32)
    for o in range(OUT):
        lwb[o] = np.repeat(lin_w[o].astype(np.float32), NBLK)[None, :]

    xts = []
    for c in range(NCORES):
        xc = xflat[c * MC : (c + 1) * MC]
        xts.append(np.ascontiguousarray(xc[rows].T))
    return xts, w3, bias36, ident36, lwb


def kernel(x, w_ih, w_hh, b_ih, b_hh, lin_w, lin_b, unused=None, **_):
    global _COMPILED
    from concourse.bass_utils import run_bass_kernel_spmd

    x = np.asarray(x, dtype=np.float32)
    w_ih = np.asarray(w_ih, dtype=np.float32)
    w_hh = np.asarray(w_hh, dtype=np.float32).reshape(-1)
    b_ih = np.asarray(b_ih, dtype=np.float32)
    b_hh = np.asarray(b_hh, dtype=np.float32)
    lin_w = np.asarray(lin_w, dtype=np.float32)
    lin_b = np.asarray(lin_b, dtype=np.float32)

    key = (w_hh.tobytes(), b_hh.tobytes(), lin_b.tobytes())
    if _COMPILED is None or _COMPILED[1] != key:
        _COMPILED = (_build_program(w_hh, b_hh, lin_b), key)
    nc = _COMPILED[0]

    xts, w3, bias36, ident36, lwb = _host_prep(x, w_ih, w_hh, b_ih, b_hh, lin_w)
    in_maps = [
        {"xt": xts[c], "w3": w3, "bias36": bias36, "ident36": ident36, "lwb": lwb}
        for c in range(NCORES)
    ]
    res = run_bass_kernel_spmd(nc, in_maps, list(range(NCORES)))
    out = np.concatenate([res.results[c]["out"] for c in range(NCORES)], axis=0)
    return out.reshape(B, N, OUT)



# revision 5
# speedup vs baseline: 1.7123x; 1.7123x over previous
"""Trainium2 Bass kernel for a hidden-size-1 GRU over M=65536 independent
sequences (T=12 steps, FE=32 features), followed by relu + linear head.

Strategy (data-parallel over 8 NeuronCores, 8192 sequences each):
  - x is cast to bf16 on the host and shipped as [3 chunks, 2 halves,
    128, 4096] so each DMA is one contiguous 1 MiB read; this halves HBM
    traffic (6.3 MiB/core) and the matmul runs at bf16 rate.
  - Gate projection with x as the STATIONARY operand: per 128-sequence
    block, lhsT = x chunk [128 rows = 4 timesteps x 32 features, 128 seqs],
    rhs = a block-diagonal bf16 weight [128, 12], so gates land directly
    as [128 seqs, 12 (t,gate)] tiles in PSUM -- no PE transposes and no
    narrow PSUM copies (the two dominant costs of the previous version).
  - Gate biases are folded into the recurrence's ScalarE activation
    immediates, so the PSUM->SBUF copy is a plain dense [128, 384] copy.
  - The GRU recurrence runs batched in 2 streams of 32 blocks (aligned
    with DMA halves) to hide cross-engine latency; per step it uses the
    form h' = n + z*(h-n) with ops split across Scalar/Vector/GpSimd.
  - relu + linear head: broadcast-multiply with a precomputed lin_w tile
    and a strided tensor_reduce over T, split across Vector and GpSimd.
"""

import numpy as np

B, N, FE, T, OUT = 32, 2048, 32, 12, 3
M = B * N
NCORES = 8
MC = M // NCORES          # 8192 sequences per core
C = FE * T                # 384 contraction length
NBLK = MC // 128          # 64 column blocks of 128 sequences
NS = 2                    # recurrence streams (= DMA halves per chunk)
SB = NBLK // NS           # 32 blocks per stream
HW = MC // NS // 128 * 128 * NS // NS // 1  # unused; kept simple below

_COMPILED = None          # (nc, weights_key)


def _build_program(w_hh, b_hh, b_ih, lin_b):
    """Build the bass program. Recurrent weights/biases are baked as
    immediates (the kernel is JIT-compiled per weight values)."""
    from contextlib import ExitStack

    import concourse.bass as bass
    import concourse.tile as tile
    from concourse import mybir

    f32 = mybir.dt.float32
    bf16 = mybir.dt.bfloat16
    AF = mybir.ActivationFunctionType
    add, mult, sub = (
        mybir.AluOpType.add,
        mybir.AluOpType.mult,
        mybir.AluOpType.subtract,
    )
    wh0, wh1, wh2 = (float(w_hh[i]) for i in range(3))
    br = float(b_ih[0] + b_hh[0])
    bz = float(b_ih[1] + b_hh[1])
    bn = float(b_ih[2])
    bhh2 = float(b_hh[2])

    nc = bass.Bass("TRN2", target_bir_lowering=False, debug=False)

    xd = nc.dram_tensor("xt", [3, NS, 128, 8192 // NS], bf16, kind="ExternalInput").ap()
    w3d = nc.dram_tensor("w3", [3, 128, 12], bf16, kind="ExternalInput").ap()
    lwd = nc.dram_tensor("lwb", [128, OUT * T * SB], f32, kind="ExternalInput").ap()
    outd = nc.dram_tensor("out", [MC, OUT], f32, kind="ExternalOutput").ap()

    with ExitStack() as ctx:
        tc = ctx.enter_context(tile.TileContext(nc))
        consts = ctx.enter_context(tc.tile_pool(name="consts", bufs=1))
        xpool = ctx.enter_context(tc.tile_pool(name="x", bufs=1))
        gpool = ctx.enter_context(tc.tile_pool(name="g", bufs=1))
        work = ctx.enter_context(tc.tile_pool(name="work", bufs=2))
        psum_gp = ctx.enter_context(tc.tile_pool(name="pgp", bufs=4, space="PSUM"))

        # --- constants (small, loaded before x on the same queue) ---
        w3_sb = consts.tile([128, 36], bf16, tag="w3sb")
        for j in range(3):
            nc.sync.dma_start(out=w3_sb[:, j * 12 : (j + 1) * 12], in_=w3d[j])
        u1_sb = {}
        for s in range(NS):
            u1_sb[s] = consts.tile([128, SB], f32, tag=f"u1_{s}", name=f"u1_{s}")
            nc.vector.memset(u1_sb[s], bhh2)
        bias_sb = consts.tile([128, 3], f32, tag="bias3")
        for i, val in enumerate([br, bz, bn]):
            nc.vector.memset(bias_sb[:, i : i + 1], val)
        b_r, b_z, b_n = (bias_sb[:, i : i + 1] for i in range(3))

        # --- x stream: 6 contiguous 1 MiB DMAs, chunk-major ---
        xs = {}
        for j in range(3):
            for s in range(NS):
                xs[(j, s)] = xpool.tile(
                    [128, 8192 // NS], bf16, tag=f"x{j}_{s}", name=f"x{j}_{s}"
                )
                nc.sync.dma_start(out=xs[(j, s)], in_=xd[j, s])

        # lin_w broadcast tile, loaded after x (needed only for the head)
        lw_sb = consts.tile([128, OUT * T * SB], f32, tag="lwsb")
        nc.sync.dma_start(out=lw_sb, in_=lwd)

        # --- gates: per (chunk, stream): 32 x (ldweights + matmul N=12),
        # then one dense [128, 384] PSUM->SBUF copy into G[s] ---
        # G[s] column layout: (t*3+gate)*SB + k  (tg-major, block-minor)
        G = {}
        for s in range(NS):
            G[s] = gpool.tile([128, 36 * SB], f32, tag=f"G{s}", name=f"G{s}")
        for j in range(3):
            for s in range(NS):
                gp = psum_gp.tile([128, SB * 12], f32, tag="gp", name=f"gp{j}_{s}")
                for b in range(SB):
                    nc.tensor.matmul(
                        gp[:, b * 12 : (b + 1) * 12],
                        lhsT=xs[(j, s)][:, b * 128 : (b + 1) * 128],
                        rhs=w3_sb[:, j * 12 : (j + 1) * 12],
                        start=True,
                        stop=True,
                    )
                G3 = G[s].rearrange("p (tg k) -> p tg k", tg=36)
                nc.scalar.activation(
                    G3[:, j * 12 : (j + 1) * 12, :],
                    gp.rearrange("p (k tg) -> p tg k", k=SB),
                    AF.Copy,
                    bias=0.0,
                )

        # --- GRU recurrence, 2 streams of [128, 32] tiles ---
        # h' = n + z*(h - n); biases folded into ACT immediates.
        H = {}
        for s in range(NS):
            H[s] = gpool.tile([128, T * SB], f32, tag=f"H{s}", name=f"H{s}")

        def gcol(s, t, gate):
            return G[s][:, (t * 3 + gate) * SB : (t * 3 + gate + 1) * SB]

        for t in range(T):
            for s in range(NS):
                nm = f"_{t}_{s}"
                ht = H[s][:, t * SB : (t + 1) * SB]
                if t == 0:
                    r = work.tile([128, SB], f32, tag="r", name="r" + nm)
                    z = work.tile([128, SB], f32, tag="z", name="z" + nm)
                    v = work.tile([128, SB], f32, tag="v", name="v" + nm)
                    w = work.tile([128, SB], f32, tag="w", name="w" + nm)
                    n = work.tile([128, SB], f32, tag="n", name="n" + nm)
                    zn = work.tile([128, SB], f32, tag="zn", name="zn" + nm)
                    nc.scalar.activation(r, gcol(s, 0, 0), AF.Sigmoid, bias=b_r)
                    nc.scalar.activation(z, gcol(s, 0, 1), AF.Sigmoid, bias=b_z)
                    nc.scalar.mul(v, r, bhh2)
                    nc.vector.tensor_add(w, v, gcol(s, 0, 2))
                    nc.scalar.activation(n, w, AF.Tanh, bias=b_n)
                    nc.gpsimd.tensor_mul(zn, z, n)
                    nc.gpsimd.tensor_tensor(ht, n, zn, sub)
                else:
                    hprev = H[s][:, (t - 1) * SB : t * SB]
                    sr = work.tile([128, SB], f32, tag="sr", name="sr" + nm)
                    sz = work.tile([128, SB], f32, tag="sz", name="sz" + nm)
                    r = work.tile([128, SB], f32, tag="r", name="r" + nm)
                    z = work.tile([128, SB], f32, tag="z", name="z" + nm)
                    u = work.tile([128, SB], f32, tag="u", name="u" + nm)
                    v = work.tile([128, SB], f32, tag="v", name="v" + nm)
                    w = work.tile([128, SB], f32, tag="w", name="w" + nm)
                    n = work.tile([128, SB], f32, tag="n", name="n" + nm)
                    d = work.tile([128, SB], f32, tag="d", name="d" + nm)
                    zd = work.tile([128, SB], f32, tag="zd", name="zd" + nm)
                    nc.vector.scalar_tensor_tensor(sr, hprev, wh0, gcol(s, t, 0), mult, add)
                    nc.vector.scalar_tensor_tensor(sz, hprev, wh1, gcol(s, t, 1), mult, add)
                    nc.scalar.activation(r, sr, AF.Sigmoid, bias=b_r)
                    nc.scalar.activation(z, sz, AF.Sigmoid, bias=b_z)
                    # u = wh2*h + bhh2 (on V to keep ScalarE at 3 ops/step)
                    nc.vector.scalar_tensor_tensor(u, hprev, wh2, u1_sb[s], mult, add)
                    nc.vector.tensor_mul(v, r, u)
                    nc.vector.tensor_add(w, v, gcol(s, t, 2))
                    nc.scalar.activation(n, w, AF.Tanh, bias=b_n)
                    nc.gpsimd.tensor_tensor(d, hprev, n, sub)
                    nc.gpsimd.tensor_mul(zd, z, d)
                    nc.gpsimd.tensor_add(ht, n, zd)

        # --- relu + linear head + output DMA, per stream ---
        outd_r = outd.rearrange("(p s k) o -> s p (k o)", p=128, s=NS)
        for s in range(NS):
            HR = gpool.tile([128, T * SB], f32, tag=f"HR{s}", name=f"HR{s}")
            for half in range(2):
                sl = slice(half * T * SB // 2, (half + 1) * T * SB // 2)
                nc.scalar.activation(HR[:, sl], H[s][:, sl], AF.Relu)
            out_sb = gpool.tile([128, SB * OUT], f32, tag=f"out{s}", name=f"out{s}")
            out_v = out_sb.rearrange("p (k o) -> p o k", o=OUT)
            for o in range(OUT):
                eng = nc.vector if o < 2 else nc.gpsimd
                P = work.tile([128, T * SB], f32, tag=f"P{o}", name=f"P{o}_{s}")
                eng.tensor_mul(P, HR, lw_sb[:, o * T * SB : (o + 1) * T * SB])
                acc = work.tile([128, SB], f32, tag=f"acc{o}", name=f"acc{o}_{s}")
                nc.vector.tensor_reduce(
                    acc,
                    P.rearrange("p (t k) -> p k t", t=T),
                    axis=mybir.AxisListType.X,
                    op=add,
                )
                eng.tensor_scalar_add(out_v[:, o, :], acc, float(lin_b[o]))
            nc.sync.dma_start(out=outd_r[s], in_=out_sb)

    _split_multi_waits(nc)
    return nc


def _split_multi_waits(nc):
    """Walrus (this build) rejects instructions with more than one sync-wait
    command. Hoist extra waits onto same-engine NoOps placed just before the
    offending instruction — the engine stream blocks on the NoOps first, so
    semantics are identical."""
    from concourse import mybir

    nid = [0]

    def fresh():
        nid[0] += 1
        return f"I-waitsplit-{nid[0]}"

    for bb in nc.main_func.blocks:
        out = []
        for insn in bb.instructions:
            si = insn.sync_info
            if si is not None and si.on_wait and len(si.on_wait) > 1:
                waits = list(si.on_wait)
                for w in waits[:-1]:
                    nop = mybir.InstNoOp(
                        name=fresh(), engine=insn.engine, ins=[], outs=[]
                    )
                    nop.sync_info = mybir.SyncInfo(on_wait=[w], on_update=[])
                    out.append(nop)
                insn.sync_info = mybir.SyncInfo(
                    on_wait=[waits[-1]], on_update=list(si.on_update or [])
                )
            out.append(insn)
        bb.instructions = out


def _host_prep(x, w_ih, lin_w):
    """Build per-core bf16 x shards and the shared constant arrays."""
    import ml_dtypes

    bf = ml_dtypes.bfloat16
    # t-major feature rows: row r = t*FE + f
    xflat = np.ascontiguousarray(
        x.reshape(M, FE, T).transpose(0, 2, 1).reshape(M, C)
    ).astype(bf)
    # column c = b*128 + p holds sequence m = p*(MC/128) + b
    cc = np.arange(MC)
    perm = (cc % 128) * NBLK + cc // 128

    # w3[j, k, tgl] = w_ih[tgl%3, k%32] when tgl//3 == k//32 else 0
    w3 = np.zeros((3, 128, 12), dtype=np.float32)
    for tl in range(4):
        w3[:, tl * 32 : (tl + 1) * 32, tl * 3 : (tl + 1) * 3] = w_ih.T[None]
    w3 = w3.astype(bf)

    # lwb col o*T*SB + t*SB + k = lin_w[o, t]
    lwb = np.empty((128, OUT * T * SB), dtype=np.float32)
    lwb[:] = np.repeat(lin_w.astype(np.float32).reshape(-1), SB)[None, :]

    xts = []
    for c in range(NCORES):
        xc = xflat[c * MC : (c + 1) * MC]
        xt = np.ascontiguousarray(xc[perm].T)          # [C, MC]
        xts.append(
            np.ascontiguousarray(
                xt.reshape(3, 128, NS, 8192 // NS).transpose(0, 2, 1, 3)
            )
        )
    return xts, w3, lwb


def kernel(x, w_ih, w_hh, b_ih, b_hh, lin_w, lin_b, unused=None, **_):
    global _COMPILED
    from concourse.bass_utils import run_bass_kernel_spmd

    x = np.asarray(x, dtype=np.float32)
    w_ih = np.asarray(w_ih, dtype=np.float32)
    w_hh = np.asarray(w_hh, dtype=np.float32).reshape(-1)
    b_ih = np.asarray(b_ih, dtype=np.float32)
    b_hh = np.asarray(b_hh, dtype=np.float32)
    lin_w = np.asarray(lin_w, dtype=np.float32)
    lin_b = np.asarray(lin_b, dtype=np.float32)

    key = (w_hh.tobytes(), b_hh.tobytes(), b_ih.tobytes(), lin_b.tobytes())
    if _COMPILED is None or _COMPILED[1] != key:
        _COMPILED = (_build_program(w_hh, b_hh, b_ih, lin_b), key)
    nc = _COMPILED[0]

    xts, w3, lwb = _host_prep(x, w_ih, lin_w)
    in_maps = [
        {"xt": xts[c], "w3": w3, "lwb": lwb} for c in range(NCORES)
    ]
    res = run_bass_kernel_spmd(nc, in_maps, list(range(NCORES)))
    out = np.concatenate([res.results[c]["out"] for c in range(NCORES)], axis=0)
    return out.reshape(B, N, OUT)
